# revision 1
# baseline (speedup 1.0000x reference)
"""Trainium2 Bass kernel for nn_BDHBlock (pre-LN latent block with
softmax-free attention and sigmoid gating).

Sharding: data-parallel over batch B=16 across 8 cores (2 per core).
No collectives; outputs are concatenated on the host.

Per-core math (B_loc=2, N=1024, D=768, H=12, HD=64), all matmuls fp16
with fp32 PSUM accumulation:
  xn   = LayerNorm(x) * ln_w + ln_b              (token-major, fp32)
  lat  = relu(xn @ enc_w.T + enc_b)              (feature-major)
  qk   = rope(lat @ qk_w.T + qk_b) / sqrt(sqrt(HD))   (token-major)
  v    = lat @ v_w.T + v_b                       (token-major)
  T_h  = qk_h^T @ v_h         per (b,h)          [HD, HD]
  attn_h = qk_h @ T_h      (== (qk qk^T/8) v by associativity)
  out  = x + sigmoid(xn @ gate_w.T + gate_b) * (attn @ out_w.T + out_b)

The softmax-free attention makes scores@v associative, so the N x N
score matrices are never materialized.
"""

import os
import sys

for _p in ("/opt/trn_rl_repo", "/root/.axon_site/_ro/trn_rl_repo"):
    if os.path.isdir(_p) and _p not in sys.path:
        sys.path.insert(0, _p)

import math
import numpy as np

import concourse.bass as bass
import concourse.mybir as mybir
from concourse import bacc
from concourse import bass_utils
from concourse.bass import ts, ds
from concourse.tile import TileContext
from concourse.masks import make_identity

F32 = mybir.dt.float32
F16 = mybir.dt.float16
AF = mybir.ActivationFunctionType

P = 128          # partitions
D = 768
KT = D // P      # 6 d-tiles
B_LOC = 2        # batch elements per core
SEQ = 1024
T = B_LOC * SEQ  # 2048 tokens per core
NT = T // P      # 16 token tiles
TPB = SEQ // P   # 8 token tiles per batch element
TW = 512         # token window (feature-major matmul free dim)
NTW = T // TW    # 4
JW = 384         # feature window (token-major matmul free dim)
NJW = D // JW    # 2
H = 12
HD = 64
EPS = 1e-5
QK_SCALE = 1.0 / math.sqrt(math.sqrt(HD))  # applied twice => 1/sqrt(HD)

# weight prep order: gate_w reuses enc_w's slot (enc phase is done by then)
W_NAMES = ["enc_w", "qk_w", "v_w", "out_w", "gate_w"]


def _trig_coefs():
    """Power-series coefficients for sin(x)=x*S(x^2), cos(x)=C(x^2) on
    |x|<=8 (the ACT Sin LUT is unusable outside a small range)."""
    xs = np.linspace(1e-8, 8.0, 40001)
    u = xs ** 2
    cheb = np.polynomial.chebyshev
    s = cheb.cheb2poly(cheb.chebfit(u, np.sin(xs) / xs, 12))
    c = cheb.cheb2poly(cheb.chebfit(u, np.cos(xs), 12))
    return [float(v) for v in s], [float(v) for v in c]


SIN_COEF, COS_COEF = _trig_coefs()


def build_nc():
    nc = bacc.Bacc("TRN2", target_bir_lowering=False, debug=False)

    x_in = nc.dram_tensor("x", [B_LOC, SEQ, D], F32, kind="ExternalInput")
    rope_in = nc.dram_tensor("rope_emb", [SEQ, HD], F32, kind="ExternalInput")
    vecs = {}
    for nm in ["ln_w", "ln_b", "enc_b", "qk_b", "v_b", "out_b", "gate_b"]:
        vecs[nm] = nc.dram_tensor(nm, [D], F32, kind="ExternalInput")
    w_in = {nm: nc.dram_tensor(nm, [D, D], F32, kind="ExternalInput")
            for nm in W_NAMES}
    out_t = nc.dram_tensor("out", [B_LOC, SEQ, D], F32, kind="ExternalOutput")

    x_flat = x_in.ap().rearrange("b n d -> (b n) d")
    out_flat = out_t.ap().rearrange("b n d -> (b n) d")

    with TileContext(nc) as tc:
        with (
            tc.tile_pool(name="consts", bufs=1) as cp,
            tc.tile_pool(name="wrot", bufs=3) as wrot,
            tc.tile_pool(name="big", bufs=4) as bigp,
            tc.tile_pool(name="work", bufs=2) as wk,
            tc.tile_pool(name="stats", bufs=2) as stp,
            tc.tile_pool(name="ropewk", bufs=2) as rwk,
            tc.tile_pool(name="tbuf", bufs=12) as tbp,
            tc.tile_pool(name="ps512", bufs=3, space="PSUM") as ps512,
            tc.tile_pool(name="ps384", bufs=3, space="PSUM") as ps384,
            tc.tile_pool(name="psX", bufs=2, space="PSUM") as psX,
        ):
            # ---------------- constants / weight prep ----------------
            with nc.named_scope("prep"):
                # rope tables: [128, TPB, 4, 32] = cosE, sinE, sinO, cosO
                rp = cp.tile([P, TPB, HD], F32, tag="ropein")
                nc.sync.dma_start(
                    rp[:], rope_in.ap().rearrange("(t p) d -> p t d", p=P))
                tabs = cp.tile([P, TPB, 4, HD // 2], F16, tag="ropetabs")
                eps_t = cp.tile([P, 1], F32, tag="epsc")
                nc.vector.memset(eps_t[:], EPS)
                # broadcast-to-all-partitions tiles for free-dim vectors
                bc = {}
                for nm in ["ln_w", "ln_b", "out_b", "gate_b"]:
                    bc[nm] = cp.tile([P, D], F16, tag=f"bc_{nm}",
                                     name=f"bc_{nm}")
                    nc.gpsimd.dma_start(
                        out=bc[nm][:],
                        in_=vecs[nm].ap()[None, :].to_broadcast((P, D)))
                # enc bias, per-partition layout [128, KT]
                encb = cp.tile([P, KT], F32, tag="encb")
                nc.sync.dma_start(
                    encb[:], vecs["enc_b"].ap().rearrange("(k p) -> p k", p=P))

                # identity for PE-mode transposes
                ident = cp.tile([P, P], F16, tag="ident")
                make_identity(nc, ident[:])

                # K=1 ones row + fp16 bias rows: folds free-dim biases into
                # the PSUM accumulation (saves a DVE drain op per tile)
                ones1 = cp.tile([1, P], F16, tag="ones1")
                nc.vector.memset(ones1[:], 1.0)
                brow = {}
                for nm in ["qk_b", "v_b"]:
                    b32 = wk.tile([1, D], F32, tag="brow32")
                    nc.sync.dma_start(b32[:], vecs[nm].ap()[None, :])
                    brow[nm] = cp.tile([1, D], F16, tag=f"brow_{nm}",
                                       name=f"brow_{nm}")
                    nc.vector.tensor_copy(brow[nm][:], b32[:])

            # xn^T: feature-major [128, KT, T]; lives until the gate matmuls
            # at the very end, so it gets its own slot outside the rotation.
            xnT = cp.tile([P, KT, T], F16, tag="xnT")

            # ---------------- LayerNorm (token-major) ----------------
            with nc.named_scope("ln"):
                for i in range(NT):
                    xt = wk.tile([P, D], F32, tag="xin")
                    nc.sync.dma_start(xt[:], x_flat[ts(i, P), :])
                    xg = xt[:].rearrange("p (s c) -> p s c", c=256)
                    stats = stp.tile([P, 3, 6], F32, tag="bnstats")
                    for s in range(3):
                        nc.vector.bn_stats(stats[:, s, :], xg[:, s, :])
                    mv = stp.tile([P, 2], F32, tag="bnmv")
                    nc.vector.bn_aggr(mv[:], stats[:])
                    rs = stp.tile([P, 1], F32, tag="rstd")
                    nc.scalar.activation(rs[:], mv[:, 1:2], AF.Sqrt,
                                         bias=eps_t[:])
                    nc.vector.reciprocal(rs[:], rs[:])
                    nb = stp.tile([P, 1], F32, tag="negmurs")
                    nc.vector.tensor_scalar(
                        nb[:], mv[:, 0:1], rs[:], -1.0,
                        op0=mybir.AluOpType.mult, op1=mybir.AluOpType.mult)
                    nc.scalar.activation(xt[:], xt[:], AF.Identity,
                                         bias=nb[:], scale=rs[:])
                    nc.gpsimd.tensor_mul(xt[:], xt[:], bc["ln_w"][:])
                    xn16 = wk.tile([P, D], F16, tag="xn16")
                    nc.gpsimd.tensor_add(xn16[:], xt[:], bc["ln_b"][:])
                    for k in range(KT):
                        ptr = psX.tile([P, P], F16, tag="psX",
                                       name=f"ptr_xn_{i}_{k}")
                        nc.tensor.transpose(ptr[:], xn16[:, ts(k, P)],
                                            ident[:])
                        nc.any.tensor_copy(xnT[:, k, ts(i, P)], ptr[:])

            with nc.named_scope("prep2"):
                # weights arrive host-transposed (W^T, [d, j] layout):
                # load fp32 rows, cast to fp16 -> wT [d(part), k, j]
                wT = {}
                for nm in W_NAMES:
                    wT[nm] = wrot.tile([P, KT, D], F16, tag="wT",
                                       name=f"wT_{nm}")
                    for k in range(KT):
                        wld = wk.tile([P, D], F32, tag="wload")
                        nc.sync.dma_start(wld[:], w_in[nm].ap()[ts(k, P), :])
                        nc.any.tensor_copy(wT[nm][:, k, :], wld[:])

                # sin/cos via fp32 Horner on DVE (ACT Sin LUT is inaccurate
                # for |x| beyond ~pi/2)
                u = cp.tile([P, TPB, HD], F32, tag="ropeu")
                nc.vector.tensor_mul(u[:], rp[:], rp[:])

                def horner(coef, out):
                    nc.vector.tensor_scalar(
                        out[:], u[:], coef[-1], coef[-2],
                        op0=mybir.AluOpType.mult, op1=mybir.AluOpType.add)
                    for cf in coef[-3::-1]:
                        nc.vector.tensor_mul(out[:], out[:], u[:])
                        nc.vector.tensor_scalar_add(out[:], out[:], cf)

                sin_a = cp.tile([P, TPB, HD], F32, tag="ropesin")
                cos_a = cp.tile([P, TPB, HD], F32, tag="ropecos")
                horner(SIN_COEF, sin_a)
                nc.vector.tensor_mul(sin_a[:], sin_a[:], rp[:])
                horner(COS_COEF, cos_a)
                nc.vector.tensor_scalar_mul(
                    tabs[:, :, 0, :], cos_a[:, :, 0::2], QK_SCALE)
                nc.vector.tensor_scalar_mul(
                    tabs[:, :, 1, :], sin_a[:, :, 0::2], QK_SCALE)
                nc.vector.tensor_scalar_mul(
                    tabs[:, :, 2, :], sin_a[:, :, 1::2], QK_SCALE)
                nc.vector.tensor_scalar_mul(
                    tabs[:, :, 3, :], cos_a[:, :, 1::2], QK_SCALE)

            # ---------------- encoder: latT = relu(Wenc @ xn^T) ------
            latT = bigp.tile([P, KT, T], F16, tag="big", name="latT")
            with nc.named_scope("enc"):
                for tw in range(NTW):
                    for j in range(KT):
                        ps = ps512.tile([P, TW], F32, tag="ps512")
                        for k in range(KT):
                            nc.tensor.matmul(
                                ps[:], wT["enc_w"][:, k, ts(j, P)],
                                xnT[:, k, ts(tw, TW)],
                                start=(k == 0), stop=(k == KT - 1))
                        nc.scalar.activation(latT[:, j, ts(tw, TW)], ps[:],
                                             AF.Relu, bias=encb[:, j:j + 1])

            # ---------------- qk (token-major) + rope ----------------
            qkR = bigp.tile([P, NT, D], F16, tag="big", name="qkR")
            qkT = bigp.tile([P, KT, T], F16, tag="big", name="qkT")
            with nc.named_scope("qk"):
                for i in range(NT):
                    ti = i % TPB
                    for jw in range(NJW):
                        ps = ps384.tile([P, JW], F32, tag="ps384")
                        for k in range(KT):
                            nc.tensor.matmul(
                                ps[:], latT[:, k, ts(i, P)],
                                wT["qk_w"][:, k, ts(jw, JW)],
                                start=(k == 0), stop=False)
                        nc.tensor.matmul(
                            ps[:], ones1[:], brow["qk_b"][:, ts(jw, JW)],
                            start=False, stop=True)
                        xb = rwk.tile([P, JW], F16, tag="ropexb")
                        nc.vector.tensor_copy(xb[:], ps[:])
                        # rope on 6 heads at once via step-0 broadcast tables
                        xbh = xb[:].rearrange("p (h d) -> p h d", d=HD)
                        x1 = xbh[:, :, 0:HD // 2]
                        x2 = xbh[:, :, HD // 2:]
                        o = qkR[:, i, ts(jw, JW)].rearrange(
                            "p (h d) -> p h d", d=HD)
                        nh = JW // HD
                        cosE = tabs[:, ti, 0, None, :].to_broadcast(
                            (P, nh, HD // 2))
                        sinE = tabs[:, ti, 1, None, :].to_broadcast(
                            (P, nh, HD // 2))
                        sinO = tabs[:, ti, 2, None, :].to_broadcast(
                            (P, nh, HD // 2))
                        cosO = tabs[:, ti, 3, None, :].to_broadcast(
                            (P, nh, HD // 2))
                        p1 = rwk.tile([P, nh, HD // 2], F16, tag="ropep1")
                        p2 = rwk.tile([P, nh, HD // 2], F16, tag="ropep2")
                        nc.any.tensor_mul(p1[:], x1, cosE)
                        nc.any.tensor_mul(p2[:], x2, sinE)
                        nc.gpsimd.tensor_sub(o[:, :, 0:HD // 2], p1[:], p2[:])
                        p3 = rwk.tile([P, nh, HD // 2], F16, tag="ropep1")
                        p4 = rwk.tile([P, nh, HD // 2], F16, tag="ropep2")
                        nc.any.tensor_mul(p3[:], x1, sinO)
                        nc.any.tensor_mul(p4[:], x2, cosO)
                        nc.gpsimd.tensor_add(o[:, :, HD // 2:], p3[:], p4[:])
                    for k in range(KT):
                        ptr = psX.tile([P, P], F16, tag="psX",
                                       name=f"ptr_qk_{i}_{k}")
                        nc.tensor.transpose(ptr[:], qkR[:, i, ts(k, P)],
                                            ident[:])
                        nc.any.tensor_copy(qkT[:, k, ts(i, P)], ptr[:])

            # ---------------- v (token-major) ------------------------
            vtm = bigp.tile([P, NT, D], F16, tag="big", name="v")
            with nc.named_scope("v"):
                for i in range(NT):
                    for jw in range(NJW):
                        ps = ps384.tile([P, JW], F32, tag="ps384")
                        for k in range(KT):
                            nc.tensor.matmul(
                                ps[:], latT[:, k, ts(i, P)],
                                wT["v_w"][:, k, ts(jw, JW)],
                                start=(k == 0), stop=False)
                        nc.tensor.matmul(
                            ps[:], ones1[:], brow["v_b"][:, ts(jw, JW)],
                            start=False, stop=True)
                        nc.any.tensor_copy(vtm[:, i, ts(jw, JW)], ps[:])

            # ---------------- attention ------------------------------
            # M1: T_h = qk_h^T @ v_h  [HD, HD] per (b, head); head pairs
            # packed into array column halves.  M2: attnT_h = T_h^T @ qkT_h.
            # All M1 products first so qkR/v are fully released before the
            # attnT slot (which reuses qkR's ring slot) is first written.
            t16s = {}
            with nc.named_scope("attn_m1"):
                for b in range(B_LOC):
                    for hp in range(KT):
                        hA, hB = 2 * hp, 2 * hp + 1
                        pt = psX.tile([P, HD], F32, tag="psX",
                                      name=f"ptm1_{b}_{hp}")
                        for m in range(TPB):
                            mt = b * TPB + m
                            nc.tensor.matmul(
                                pt[0:HD, :],
                                qkR[:, mt, ts(hA, HD)], vtm[:, mt, ts(hA, HD)],
                                start=(m == 0), stop=(m == TPB - 1),
                                tile_position=(0, 0))
                            nc.tensor.matmul(
                                pt[HD:P, :],
                                qkR[:, mt, ts(hB, HD)], vtm[:, mt, ts(hB, HD)],
                                start=(m == 0), stop=(m == TPB - 1),
                                tile_position=(0, HD))
                        t16 = tbp.tile([P, HD], F16, tag="t16",
                                       name=f"t16_{b}_{hp}")
                        nc.scalar.activation(t16[:], pt[:], AF.Copy)
                        t16s[(b, hp)] = t16

            attnT = bigp.tile([P, KT, T], F16, tag="big", name="attnT")
            with nc.named_scope("attn_m2"):
                for b in range(B_LOC):
                    for hp in range(KT):
                        t16 = t16s[(b, hp)]
                        for nw in range(2):
                            col = b * SEQ + nw * TW
                            ps = ps512.tile([P, TW], F32, tag="ps512")
                            nc.tensor.matmul(
                                ps[0:HD, :], t16[0:HD, :],
                                qkT[0:HD, hp, ds(col, TW)],
                                start=True, stop=True, tile_position=(0, 0))
                            nc.tensor.matmul(
                                ps[HD:P, :], t16[HD:P, :],
                                qkT[HD:P, hp, ds(col, TW)],
                                start=True, stop=True, tile_position=(HD, HD))
                            nc.any.tensor_copy(attnT[:, hp, ds(col, TW)],
                                               ps[:])

            # ------------- gate + output projection + residual -------
            with nc.named_scope("out"):
                for i in range(NT):
                    xr = wk.tile([P, D], F32, tag="xres")
                    nc.sync.dma_start(xr[:], x_flat[ts(i, P), :])
                    for jw in range(NJW):
                        psg = ps384.tile([P, JW], F32, tag="ps384")
                        for k in range(KT):
                            nc.tensor.matmul(
                                psg[:], xnT[:, k, ts(i, P)],
                                wT["gate_w"][:, k, ts(jw, JW)],
                                start=(k == 0), stop=(k == KT - 1))
                        gt = rwk.tile([P, JW], F16, tag="ropexb")
                        nc.vector.tensor_add(
                            gt[:], psg[:], bc["gate_b"][:, ts(jw, JW)])
                        g16 = rwk.tile([P, JW], F16, tag="g16")
                        nc.scalar.activation(g16[:], gt[:], AF.Sigmoid)

                        ps = ps384.tile([P, JW], F32, tag="ps384")
                        for k in range(KT):
                            nc.tensor.matmul(
                                ps[:], attnT[:, k, ts(i, P)],
                                wT["out_w"][:, k, ts(jw, JW)],
                                start=(k == 0), stop=(k == KT - 1))
                        ao = wk.tile([P, JW], F32, tag="xn16")
                        nc.vector.tensor_add(
                            ao[:], ps[:], bc["out_b"][:, ts(jw, JW)])
                        nc.vector.tensor_mul(ao[:], ao[:], g16[:])
                        nc.gpsimd.tensor_add(xr[:, ds(jw * JW, JW)], ao[:],
                                             xr[:, ds(jw * JW, JW)])
                    nc.sync.dma_start(out_flat[ts(i, P), :], xr[:])

    nc.finalize()
    return nc


_NC = None


def _get_nc():
    global _NC
    if _NC is None:
        _NC = build_nc()
    return _NC


def make_in_maps(inputs, n_cores=8):
    x = np.ascontiguousarray(inputs["x"], dtype=np.float32)
    shared = {}
    for nm in ["rope_emb", "ln_w", "ln_b", "enc_b", "qk_b", "v_b", "out_b",
               "gate_b"]:
        shared[nm] = np.ascontiguousarray(inputs[nm], dtype=np.float32)
    # per-head output-feature permutation (evens then odds) makes the
    # on-device rope slices contiguous; pure layout prep
    perm = np.concatenate(
        [h * HD + np.concatenate([np.arange(0, HD, 2), np.arange(1, HD, 2)])
         for h in range(H)])
    shared["qk_b"] = np.ascontiguousarray(shared["qk_b"][perm])
    for nm in W_NAMES:
        # device consumes W^T ([d, j]); transpose is host-side layout prep
        w = np.asarray(inputs[nm], dtype=np.float32)
        if nm == "qk_w":
            w = w[perm]
        shared[nm] = np.ascontiguousarray(w.T)
    in_maps = []
    for c in range(n_cores):
        m = dict(shared)
        m["x"] = np.ascontiguousarray(x[c * B_LOC:(c + 1) * B_LOC])
        in_maps.append(m)
    return in_maps


def kernel(**inputs):
    nc = _get_nc()
    n_cores = 8
    in_maps = make_in_maps(inputs, n_cores)
    res = bass_utils.run_bass_kernel_spmd(
        nc, in_maps, core_ids=list(range(n_cores)))
    return np.concatenate([r["out"] for r in res.results], axis=0)



# revision 3
# speedup vs baseline: 1.0881x; 1.0881x over previous
"""Trainium2 Bass kernel for nn_BDHBlock (pre-LN latent block with
softmax-free attention and sigmoid gating).

Sharding: data-parallel over batch B=16 across 8 cores (2 per core).
No collectives; outputs are concatenated on the host.

Per-core math (B_loc=2, N=1024, D=768, H=12, HD=64), all matmuls fp16
with fp32 PSUM accumulation:
  z    = (x - mu) * rstd                          (token-major, fp32)
  lat  = relu(z @ enc_w'.T + enc_b')              (feature-major)
  qk   = rope(lat @ qk_w.T + qk_b) / sqrt(sqrt(HD))   (token-major)
  v    = lat @ v_w.T + v_b                        (token-major)
  T_h  = qk_h^T @ v_h         per (b,h)           [HD, HD]
  attn_h = qk_h @ T_h      (== (qk qk^T/8) v by associativity)
  out  = x + sigmoid(z @ gate_w'.T + gate_b') * (attn @ out_w.T + out_b)

where enc_w' = enc_w*diag(ln_w), enc_b' = enc_b + enc_w@ln_b (and same
for gate) fold the LayerNorm affine into the weights host-side.
Weights arrive host-transposed and fp16-cast; rope cos/sin tables are
precomputed on the host from rope_emb.  The softmax-free attention
makes scores@v associative, so the N x N score matrices are never
materialized.  All feature-major transposes (xn^T, qk^T) go through the
DMA XBAR transpose engine, keeping PE free for real matmuls.
"""

import os
import sys

for _p in ("/opt/trn_rl_repo", "/root/.axon_site/_ro/trn_rl_repo"):
    if os.path.isdir(_p) and _p not in sys.path:
        sys.path.insert(0, _p)

import math
import numpy as np

import concourse.bass as bass
import concourse.mybir as mybir
from concourse import bacc
from concourse import bass_utils
from concourse.bass import ts, ds
from concourse.tile import TileContext

F32 = mybir.dt.float32
F16 = mybir.dt.float16
AF = mybir.ActivationFunctionType

P = 128          # partitions
D = 768
KT = D // P      # 6 d-tiles
B_LOC = 2        # batch elements per core
SEQ = 1024
T = B_LOC * SEQ  # 2048 tokens per core
NT = T // P      # 16 token tiles
TPB = SEQ // P   # 8 token tiles per batch element
TW = 512         # token window (feature-major matmul free dim)
NTW = T // TW    # 4
JW = 384         # feature window (token-major matmul free dim)
NJW = D // JW    # 2
H = 12
HD = 64
EPS = 1e-5
QK_SCALE = 1.0 / math.sqrt(math.sqrt(HD))  # applied twice => 1/sqrt(HD)
RB = 4           # token tiles per rope batch
NH = RB * H      # fused (tile, head) rope rows: stride over D is uniform

W_NAMES = ["enc_w", "qk_w", "v_w", "out_w", "gate_w"]
# bvec rows: broadcast free-dim bias vectors
BV_QK, BV_V, BV_OUT, BV_GATE = 0, 1, 2, 3


def build_nc():
    nc = bacc.Bacc("TRN2", target_bir_lowering=False, debug=False)

    x_in = nc.dram_tensor("x", [B_LOC, SEQ, D], F32, kind="ExternalInput")
    tabs_in = nc.dram_tensor("rope_tabs", [P, TPB, 4, HD // 2], F16,
                             kind="ExternalInput")
    encb_in = nc.dram_tensor("encb", [P, KT], F32, kind="ExternalInput")
    bvec_in = nc.dram_tensor("bvecs", [P, 4, D], F16, kind="ExternalInput")
    w_in = {nm: nc.dram_tensor(nm, [D, D], F16, kind="ExternalInput")
            for nm in W_NAMES}
    out_t = nc.dram_tensor("out", [B_LOC, SEQ, D], F32, kind="ExternalOutput")

    x_flat = x_in.ap().rearrange("b n d -> (b n) d")
    out_flat = out_t.ap().rearrange("b n d -> (b n) d")

    with TileContext(nc) as tc:
        with (
            tc.tile_pool(name="consts", bufs=1) as cp,
            tc.tile_pool(name="wrot", bufs=3) as wrot,
            tc.tile_pool(name="big", bufs=4) as bigp,
            tc.tile_pool(name="work", bufs=2) as wk,
            tc.tile_pool(name="stats", bufs=2) as stp,
            tc.tile_pool(name="ropewk", bufs=2) as rwk,
            tc.tile_pool(name="qraw", bufs=2) as qrp,
            tc.tile_pool(name="gwk", bufs=2) as gwk,
            tc.tile_pool(name="tbuf", bufs=12) as tbp,
            tc.tile_pool(name="psA", bufs=4, space="PSUM") as psA,
            tc.tile_pool(name="psB", bufs=4, space="PSUM") as psB,
        ):
            # ---------------- constants / weight prep ----------------
            with nc.named_scope("prep"):
                eps_t = cp.tile([P, 1], F32, tag="epsc")
                nc.vector.memset(eps_t[:], EPS)
                # rope tables: [128, TPB, 4, 32] = cosE, sinE, sinO, cosO
                tabs = cp.tile([P, TPB, 4, HD // 2], F16, tag="ropetabs")
                nc.sync.dma_start(tabs[:], tabs_in.ap())
                # enc bias (folded), per-partition layout [128, KT]
                encb = cp.tile([P, KT], F32, tag="encb")
                nc.sync.dma_start(encb[:], encb_in.ap())
                # pre-broadcast free-dim bias rows (qk_b, v_b, out_b, gate_b)
                bvec = cp.tile([P, 4, D], F16, tag="bvec")
                nc.sync.dma_start(bvec[:], bvec_in.ap())

            # xn^T: feature-major [128, KT, T]; lives until the gate matmuls
            xnT = cp.tile([P, KT, T], F16, tag="xnT")

            # weights host-prepped: fp16 W^T (LN affine folded into
            # enc/gate); 5 logical tiles rotate through 3 slots
            wT = {}
            for nm in ["enc_w", "qk_w", "v_w"]:
                wT[nm] = wrot.tile([P, KT, D], F16, tag="wT", name=f"wT_{nm}")
                nc.sync.dma_start(
                    wT[nm][:], w_in[nm].ap().rearrange("(k p) j -> p k j", p=P))

            # ---------------- LayerNorm (token-major) ----------------
            with nc.named_scope("ln"):
                for i in range(NT):
                    xt = wk.tile([P, D], F32, tag="xin")
                    nc.sync.dma_start(xt[:], x_flat[ts(i, P), :])
                    xg = xt[:].rearrange("p (s c) -> p s c", c=256)
                    stats = stp.tile([P, 3, 6], F32, tag="bnstats")
                    for s in range(3):
                        nc.vector.bn_stats(stats[:, s, :], xg[:, s, :])
                    mv = stp.tile([P, 2], F32, tag="bnmv")
                    nc.vector.bn_aggr(mv[:], stats[:])
                    rs = stp.tile([P, 1], F32, tag="rstd")
                    nc.scalar.activation(rs[:], mv[:, 1:2], AF.Sqrt,
                                         bias=eps_t[:])
                    nc.vector.reciprocal(rs[:], rs[:])
                    nb = stp.tile([P, 1], F32, tag="negmurs")
                    nc.vector.tensor_scalar(
                        nb[:], mv[:, 0:1], rs[:], -1.0,
                        op0=mybir.AluOpType.mult, op1=mybir.AluOpType.mult)
                    xn16 = wk.tile([P, D], F16, tag="xn16")
                    nc.scalar.activation(xn16[:], xt[:], AF.Identity,
                                         bias=nb[:], scale=rs[:])
                    # feature-major via DMA XBAR transpose (PE stays free)
                    nc.sync.dma_start(xnT[:, :, ts(i, P)], xn16[:],
                                      transpose=True)

            # ---------------- encoder: latT = relu(Wenc @ xn^T) ------
            latT = bigp.tile([P, KT, T], F16, tag="big", name="latT")
            with nc.named_scope("enc"):
                for tw in range(NTW):
                    for j in range(KT):
                        ps = psA.tile([P, TW], F32, tag="psA")
                        for k in range(KT):
                            nc.tensor.matmul(
                                ps[:], wT["enc_w"][:, k, ts(j, P)],
                                xnT[:, k, ts(tw, TW)],
                                start=(k == 0), stop=(k == KT - 1))
                        nc.scalar.activation(latT[:, j, ts(tw, TW)], ps[:],
                                             AF.Relu, bias=encb[:, j:j + 1])

            # late weights reuse the first two wrot slots (deps auto-wait)
            for nm in ["out_w", "gate_w"]:
                wT[nm] = wrot.tile([P, KT, D], F16, tag="wT", name=f"wT_{nm}")
                nc.sync.dma_start(
                    wT[nm][:], w_in[nm].ap().rearrange("(k p) j -> p k j", p=P))

            # ---------------- qk (token-major) + rope ----------------
            qkR = bigp.tile([P, NT, D], F16, tag="big", name="qkR")
            qkT = bigp.tile([P, KT, T], F16, tag="big", name="qkT")
            with nc.named_scope("qk"):
                for g in range(NT // RB):
                    qraw = qrp.tile([P, RB, D], F16, tag="qraw")
                    for r in range(RB):
                        i = g * RB + r
                        for jw in range(NJW):
                            ps = psB.tile([P, JW], F32, tag="psB")
                            for k in range(KT):
                                nc.tensor.matmul(
                                    ps[:], latT[:, k, ts(i, P)],
                                    wT["qk_w"][:, k, ts(jw, JW)],
                                    start=(k == 0), stop=(k == KT - 1))
                            nc.vector.tensor_add(
                                qraw[:, r, ts(jw, JW)], ps[:],
                                bvec[:, BV_QK, ts(jw, JW)])
                    # rope on RB tiles x 12 heads in single big 4D ops
                    ti0 = (g * RB) % TPB
                    xf = qraw[:].rearrange("p t (f d) -> p t f d", d=HD)
                    x1 = xf[:, :, :, 0:HD // 2]
                    x2 = xf[:, :, :, HD // 2:]
                    of = qkR[:, ds(g * RB, RB), :].rearrange(
                        "p t (f d) -> p t f d", d=HD)
                    tb = [tabs[:, ds(ti0, RB), c, None, :].to_broadcast(
                        (P, RB, H, HD // 2)) for c in range(4)]
                    p1 = rwk.tile([P, RB, H, HD // 2], F16, tag="ropep1")
                    p2 = rwk.tile([P, RB, H, HD // 2], F16, tag="ropep2")
                    nc.vector.tensor_mul(p1[:], x1, tb[0])
                    nc.vector.tensor_mul(p2[:], x2, tb[1])
                    p3 = rwk.tile([P, RB, H, HD // 2], F16, tag="ropep1")
                    p4 = rwk.tile([P, RB, H, HD // 2], F16, tag="ropep2")
                    nc.vector.tensor_mul(p3[:], x1, tb[2])
                    nc.vector.tensor_mul(p4[:], x2, tb[3])
                    nc.gpsimd.tensor_sub(of[:, :, :, 0:HD // 2], p1[:], p2[:])
                    nc.gpsimd.tensor_add(of[:, :, :, HD // 2:], p3[:], p4[:])
                    # feature-major copy via DMA XBAR transpose
                    for r in range(RB):
                        i = g * RB + r
                        nc.sync.dma_start(qkT[:, :, ts(i, P)], qkR[:, i, :],
                                          transpose=True)

            # ---------------- v (token-major) ------------------------
            vtm = bigp.tile([P, NT, D], F16, tag="big", name="v")
            with nc.named_scope("v"):
                for i in range(NT):
                    for jw in range(NJW):
                        ps = psB.tile([P, JW], F32, tag="psB")
                        for k in range(KT):
                            nc.tensor.matmul(
                                ps[:], latT[:, k, ts(i, P)],
                                wT["v_w"][:, k, ts(jw, JW)],
                                start=(k == 0), stop=(k == KT - 1))
                        nc.vector.tensor_add(vtm[:, i, ts(jw, JW)], ps[:],
                                             bvec[:, BV_V, ts(jw, JW)])

            # ---------------- attention ------------------------------
            # M1: T_h = qk_h^T @ v_h  [HD, HD] per (b, head); head pairs
            # packed into array column halves.  M2: attnT_h = T_h^T @ qkT_h.
            t16s = {}
            with nc.named_scope("attn_m1"):
                for b in range(B_LOC):
                    for hp in range(KT):
                        hA, hB = 2 * hp, 2 * hp + 1
                        pt = psB.tile([P, HD], F32, tag="psB",
                                      name=f"ptm1_{b}_{hp}")
                        for m in range(TPB):
                            mt = b * TPB + m
                            nc.tensor.matmul(
                                pt[0:HD, :],
                                qkR[:, mt, ts(hA, HD)], vtm[:, mt, ts(hA, HD)],
                                start=(m == 0), stop=(m == TPB - 1),
                                tile_position=(0, 0))
                            nc.tensor.matmul(
                                pt[HD:P, :],
                                qkR[:, mt, ts(hB, HD)], vtm[:, mt, ts(hB, HD)],
                                start=(m == 0), stop=(m == TPB - 1),
                                tile_position=(0, HD))
                        t16 = tbp.tile([P, HD], F16, tag="t16",
                                       name=f"t16_{b}_{hp}")
                        nc.scalar.activation(t16[:], pt[:], AF.Copy)
                        t16s[(b, hp)] = t16

            attnT = bigp.tile([P, KT, T], F16, tag="big", name="attnT")
            with nc.named_scope("attn_m2"):
                for b in range(B_LOC):
                    for hp in range(KT):
                        t16 = t16s[(b, hp)]
                        for nw in range(2):
                            col = b * SEQ + nw * TW
                            ps = psA.tile([P, TW], F32, tag="psA")
                            nc.tensor.matmul(
                                ps[0:HD, :], t16[0:HD, :],
                                qkT[0:HD, hp, ds(col, TW)],
                                start=True, stop=True, tile_position=(0, 0))
                            nc.tensor.matmul(
                                ps[HD:P, :], t16[HD:P, :],
                                qkT[HD:P, hp, ds(col, TW)],
                                start=True, stop=True, tile_position=(HD, HD))
                            nc.scalar.activation(attnT[:, hp, ds(col, TW)],
                                                 ps[:], AF.Copy)

            # ------------- gate + output projection + residual -------
            with nc.named_scope("out"):
                for i in range(NT):
                    xr = wk.tile([P, D], F32, tag="xres")
                    nc.sync.dma_start(xr[:], x_flat[ts(i, P), :])
                    for jw in range(NJW):
                        psg = psB.tile([P, JW], F32, tag="psB")
                        for k in range(KT):
                            nc.tensor.matmul(
                                psg[:], xnT[:, k, ts(i, P)],
                                wT["gate_w"][:, k, ts(jw, JW)],
                                start=(k == 0), stop=(k == KT - 1))
                        gt = gwk.tile([P, JW], F16, tag="gt")
                        nc.vector.tensor_add(
                            gt[:], psg[:], bvec[:, BV_GATE, ts(jw, JW)])
                        g16 = gwk.tile([P, JW], F16, tag="g16")
                        nc.scalar.activation(g16[:], gt[:], AF.Sigmoid)

                        ps = psB.tile([P, JW], F32, tag="psB")
                        for k in range(KT):
                            nc.tensor.matmul(
                                ps[:], attnT[:, k, ts(i, P)],
                                wT["out_w"][:, k, ts(jw, JW)],
                                start=(k == 0), stop=(k == KT - 1))
                        ao = gwk.tile([P, JW], F16, tag="ao")
                        nc.vector.tensor_add(
                            ao[:], ps[:], bvec[:, BV_OUT, ts(jw, JW)])
                        nc.vector.tensor_mul(ao[:], ao[:], g16[:])
                        nc.gpsimd.tensor_add(xr[:, ds(jw * JW, JW)], ao[:],
                                             xr[:, ds(jw * JW, JW)])
                    nc.sync.dma_start(out_flat[ts(i, P), :], xr[:])

    nc.finalize()
    return nc


_NC = None


def _get_nc():
    global _NC
    if _NC is None:
        _NC = build_nc()
    return _NC


def make_in_maps(inputs, n_cores=8):
    f32 = np.float32
    x = np.ascontiguousarray(inputs["x"], dtype=f32)
    ln_w = np.asarray(inputs["ln_w"], dtype=f32)
    ln_b = np.asarray(inputs["ln_b"], dtype=f32)

    # per-head output-feature permutation (evens then odds) makes the
    # on-device rope slices contiguous; pure layout prep
    perm = np.concatenate(
        [h * HD + np.concatenate([np.arange(0, HD, 2), np.arange(1, HD, 2)])
         for h in range(H)])

    shared = {}
    # weights: fold LN affine into enc/gate, transpose, cast fp16
    wmat = {nm: np.asarray(inputs[nm], dtype=f32) for nm in W_NAMES}
    wmat["enc_w"] = wmat["enc_w"] * ln_w[None, :]
    wmat["gate_w"] = wmat["gate_w"] * ln_w[None, :]
    wmat["qk_w"] = wmat["qk_w"][perm]
    for nm in W_NAMES:
        shared[nm] = np.ascontiguousarray(wmat[nm].T.astype(np.float16))

    enc_w = np.asarray(inputs["enc_w"], dtype=f32)
    gate_w = np.asarray(inputs["gate_w"], dtype=f32)
    encb = np.asarray(inputs["enc_b"], dtype=f32) + enc_w @ ln_b
    shared["encb"] = np.ascontiguousarray(encb.reshape(KT, P).T)
    gate_b = np.asarray(inputs["gate_b"], dtype=f32) + gate_w @ ln_b

    bvecs = np.stack([
        np.asarray(inputs["qk_b"], dtype=f32)[perm],
        np.asarray(inputs["v_b"], dtype=f32),
        np.asarray(inputs["out_b"], dtype=f32),
        gate_b,
    ]).astype(np.float16)
    shared["bvecs"] = np.ascontiguousarray(
        np.broadcast_to(bvecs[None], (P, 4, D)))

    # rope tables from rope_emb: host trig, fp16, evens/odds split,
    # pre-scaled so the qk.qk^T product carries 1/sqrt(HD)
    ang = np.asarray(inputs["rope_emb"], dtype=np.float64)[:, :HD]
    cos, sin = np.cos(ang) * QK_SCALE, np.sin(ang) * QK_SCALE
    tabs = np.stack([cos[:, 0::2], sin[:, 0::2], sin[:, 1::2], cos[:, 1::2]],
                    axis=1)                          # [N, 4, 32]
    tabs = tabs.reshape(TPB, P, 4, HD // 2).transpose(1, 0, 2, 3)
    shared["rope_tabs"] = np.ascontiguousarray(tabs.astype(np.float16))

    in_maps = []
    for c in range(n_cores):
        m = dict(shared)
        m["x"] = np.ascontiguousarray(x[c * B_LOC:(c + 1) * B_LOC])
        in_maps.append(m)
    return in_maps


def kernel(**inputs):
    nc = _get_nc()
    n_cores = 8
    in_maps = make_in_maps(inputs, n_cores)
    res = bass_utils.run_bass_kernel_spmd(
        nc, in_maps, core_ids=list(range(n_cores)))
    return np.concatenate([r["out"] for r in res.results], axis=0)


# revision 5
# speedup vs baseline: 1.2271x; 1.1278x over previous
"""Trainium2 Bass kernel for nn_BDHBlock (pre-LN latent block with
softmax-free attention and sigmoid gating).

Sharding: data-parallel over batch B=16 across 8 cores (2 per core).
No collectives; outputs are concatenated on the host.

Per-core math (B_loc=2, N=1024, D=768, H=12, HD=64), all matmuls fp16
with fp32 PSUM accumulation:
  z    = (x - mu) * rstd                          (token-major, fp32)
  lat  = relu(z @ enc_w'.T + enc_b')              (feature-major)
  qk   = rope(lat @ qk_w.T + qk_b) / sqrt(sqrt(HD))   (token-major)
  v    = lat @ v_w.T + v_b                        (token-major)
  T_h  = qk_h^T @ v_h         per (b,h)           [HD, HD]
  attn_h = qk_h @ T_h      (== (qk qk^T/8) v by associativity)
  out  = x + sigmoid(z @ gate_w'.T + gate_b') * (attn @ out_w.T + out_b)

where enc_w' = enc_w*diag(ln_w), enc_b' = enc_b + enc_w@ln_b (and same
for gate) fold the LayerNorm affine into the weights host-side.
Weights arrive host-transposed and fp16-cast in a flat per-partition
layout, loaded through the gpsimd SWDGE ring so they never block the
sync ring that feeds x tiles.  Rope cos/sin tables are precomputed on
the host from rope_emb.  xn^T transposes run on PE (idle during LN);
qk^T goes through the DMA XBAR transpose engine while PE is busy with
matmuls.  The softmax-free attention makes scores@v associative, so
the N x N score matrices are never materialized.
"""

import os
import sys

for _p in ("/opt/trn_rl_repo", "/root/.axon_site/_ro/trn_rl_repo"):
    if os.path.isdir(_p) and _p not in sys.path:
        sys.path.insert(0, _p)

import math
import numpy as np

import concourse.bass as bass
import concourse.mybir as mybir
from concourse import bacc
from concourse import bass_utils
from concourse.bass import ts, ds
from concourse.tile import TileContext
from concourse.masks import make_identity

F32 = mybir.dt.float32
F16 = mybir.dt.float16
AF = mybir.ActivationFunctionType

P = 128          # partitions
D = 768
KT = D // P      # 6 d-tiles
B_LOC = 2        # batch elements per core
SEQ = 1024
T = B_LOC * SEQ  # 2048 tokens per core
NT = T // P      # 16 token tiles
TPB = SEQ // P   # 8 token tiles per batch element
TW = 512         # token window (feature-major matmul free dim)
NTW = T // TW    # 4
JW = 384         # feature window (token-major matmul free dim)
NJW = D // JW    # 2
H = 12
HD = 64
EPS = 1e-5
QK_SCALE = 1.0 / math.sqrt(math.sqrt(HD))  # applied twice => 1/sqrt(HD)
RB = 4           # token tiles per rope batch

W_NAMES = ["enc_w", "qk_w", "v_w", "out_w", "gate_w"]
# bvec rows: broadcast free-dim bias vectors
BV_QK, BV_V, BV_OUT, BV_GATE = 0, 1, 2, 3


def build_nc():
    nc = bacc.Bacc("TRN2", target_bir_lowering=False, debug=False)

    x_in = nc.dram_tensor("x", [B_LOC, SEQ, D], F32, kind="ExternalInput")
    tabs_in = nc.dram_tensor("rope_tabs", [P, TPB, 4, HD // 2], F16,
                             kind="ExternalInput")
    encb_in = nc.dram_tensor("encb", [P, KT], F32, kind="ExternalInput")
    bvec_in = nc.dram_tensor("bvecs", [P, 4, D], F16, kind="ExternalInput")
    w_in = {nm: nc.dram_tensor(nm, [P, KT * D], F16, kind="ExternalInput")
            for nm in W_NAMES}
    out_t = nc.dram_tensor("out", [B_LOC, SEQ, D], F32, kind="ExternalOutput")

    x_flat = x_in.ap().rearrange("b n d -> (b n) d")
    out_flat = out_t.ap().rearrange("b n d -> (b n) d")

    with TileContext(nc) as tc:
        with (
            tc.tile_pool(name="consts", bufs=1) as cp,
            tc.tile_pool(name="wrot", bufs=3) as wrot,
            tc.tile_pool(name="big", bufs=4) as bigp,
            tc.tile_pool(name="xload", bufs=4) as xlp,
            tc.tile_pool(name="work", bufs=2) as wk,
            tc.tile_pool(name="stats", bufs=4) as stp,
            tc.tile_pool(name="ropewk", bufs=2) as rwk,
            tc.tile_pool(name="qraw", bufs=2) as qrp,
            tc.tile_pool(name="gwk", bufs=2) as gwk,
            tc.tile_pool(name="tbuf", bufs=12) as tbp,
            tc.tile_pool(name="psA", bufs=3, space="PSUM") as psA,
            tc.tile_pool(name="psB", bufs=3, space="PSUM") as psB,
            tc.tile_pool(name="psT", bufs=2, space="PSUM") as psT,
        ):
            # ---------------- constants / weight prep ----------------
            with nc.named_scope("prep"):
                eps_t = cp.tile([P, 1], F32, tag="epsc")
                nc.vector.memset(eps_t[:], EPS)
                # rope tables: [128, TPB, 4, 32] = cosE, sinE, sinO, cosO
                tabs = cp.tile([P, TPB, 4, HD // 2], F16, tag="ropetabs")
                nc.sync.dma_start(tabs[:], tabs_in.ap())
                # enc bias (folded), per-partition layout [128, KT]
                encb = cp.tile([P, KT], F32, tag="encb")
                nc.sync.dma_start(encb[:], encb_in.ap())
                # pre-broadcast free-dim bias rows (qk_b, v_b, out_b, gate_b)
                bvec = cp.tile([P, 4, D], F16, tag="bvec")
                nc.sync.dma_start(bvec[:], bvec_in.ap())
                # identity for PE-mode transposes
                ident = cp.tile([P, P], F16, tag="ident")
                make_identity(nc, ident[:])

            # xn^T: feature-major [128, KT, T]; lives until the gate matmuls
            xnT = cp.tile([P, KT, T], F16, tag="xnT")

            # weights host-prepped: fp16 W^T (LN affine folded into
            # enc/gate) in flat [p, k*D] layout; 5 logical tiles rotate
            # through 3 slots; SWDGE ring keeps them off the sync ring
            wT = {}
            for nm in ["enc_w", "qk_w", "v_w"]:
                wT[nm] = wrot.tile([P, KT, D], F16, tag="wT", name=f"wT_{nm}")
                nc.gpsimd.dma_start(wT[nm][:], w_in[nm].ap())

            # ---------------- LayerNorm (token-major) ----------------
            with nc.named_scope("ln"):
                for i in range(NT):
                    xt = xlp.tile([P, D], F32, tag="xin")
                    nc.sync.dma_start(xt[:], x_flat[ts(i, P), :])
                    xg = xt[:].rearrange("p (s c) -> p s c", c=256)
                    stats = stp.tile([P, 3, 6], F32, tag="bnstats")
                    for s in range(3):
                        nc.vector.bn_stats(stats[:, s, :], xg[:, s, :])
                    mv = stp.tile([P, 2], F32, tag="bnmv")
                    nc.vector.bn_aggr(mv[:], stats[:])
                    rs = stp.tile([P, 1], F32, tag="rstd")
                    nc.scalar.activation(rs[:], mv[:, 1:2], AF.Sqrt,
                                         bias=eps_t[:])
                    nc.vector.reciprocal(rs[:], rs[:])
                    nb = stp.tile([P, 1], F32, tag="negmurs")
                    nc.vector.tensor_scalar(
                        nb[:], mv[:, 0:1], rs[:], -1.0,
                        op0=mybir.AluOpType.mult, op1=mybir.AluOpType.mult)
                    xn16 = wk.tile([P, D], F16, tag="xn16")
                    nc.scalar.activation(xn16[:], xt[:], AF.Identity,
                                         bias=nb[:], scale=rs[:])
                    # feature-major via PE transposes (PE is idle here),
                    # batched into one PSUM bank + one DVE drain
                    pt = psT.tile([P, D], F16, tag="psT")
                    for k in range(KT):
                        nc.tensor.transpose(pt[:, ts(k, P)], xn16[:, ts(k, P)],
                                            ident[:])
                    nc.vector.tensor_copy(
                        xnT[:, :, ts(i, P)],
                        pt[:].rearrange("p (k c) -> p k c", c=P))

            # ---------------- encoder: latT = relu(Wenc @ xn^T) ------
            latT = bigp.tile([P, KT, T], F16, tag="big", name="latT")
            with nc.named_scope("enc"):
                for tw in range(NTW):
                    for j in range(KT):
                        ps = psA.tile([P, TW], F32, tag="psA")
                        for k in range(KT):
                            nc.tensor.matmul(
                                ps[:], wT["enc_w"][:, k, ts(j, P)],
                                xnT[:, k, ts(tw, TW)],
                                start=(k == 0), stop=(k == KT - 1))
                        nc.scalar.activation(latT[:, j, ts(tw, TW)], ps[:],
                                             AF.Relu, bias=encb[:, j:j + 1])

            # late weights reuse the first two wrot slots (deps auto-wait)
            for nm in ["out_w", "gate_w"]:
                wT[nm] = wrot.tile([P, KT, D], F16, tag="wT", name=f"wT_{nm}")
                nc.gpsimd.dma_start(wT[nm][:], w_in[nm].ap())

            # ---------------- qk (token-major) + rope ----------------
            qkR = bigp.tile([P, NT, D], F16, tag="big", name="qkR")
            qkT = bigp.tile([P, KT, T], F16, tag="big", name="qkT")
            with nc.named_scope("qk"):
                for g in range(NT // RB):
                    qraw = qrp.tile([P, RB, D], F16, tag="qraw")
                    for r in range(RB):
                        i = g * RB + r
                        for jw in range(NJW):
                            ps = psB.tile([P, JW], F32, tag="psB")
                            for k in range(KT):
                                nc.tensor.matmul(
                                    ps[:], latT[:, k, ts(i, P)],
                                    wT["qk_w"][:, k, ts(jw, JW)],
                                    start=(k == 0), stop=(k == KT - 1))
                            nc.vector.tensor_add(
                                qraw[:, r, ts(jw, JW)], ps[:],
                                bvec[:, BV_QK, ts(jw, JW)])
                    # rope on RB tiles x 12 heads in single big 4D ops
                    ti0 = (g * RB) % TPB
                    xf = qraw[:].rearrange("p t (f d) -> p t f d", d=HD)
                    x1 = xf[:, :, :, 0:HD // 2]
                    x2 = xf[:, :, :, HD // 2:]
                    of = qkR[:, ds(g * RB, RB), :].rearrange(
                        "p t (f d) -> p t f d", d=HD)
                    tb = [tabs[:, ds(ti0, RB), c, None, :].to_broadcast(
                        (P, RB, H, HD // 2)) for c in range(4)]
                    p1 = rwk.tile([P, RB, H, HD // 2], F16, tag="ropep1")
                    p2 = rwk.tile([P, RB, H, HD // 2], F16, tag="ropep2")
                    nc.vector.tensor_mul(p1[:], x1, tb[0])
                    nc.vector.tensor_mul(p2[:], x2, tb[1])
                    p3 = rwk.tile([P, RB, H, HD // 2], F16, tag="ropep1")
                    p4 = rwk.tile([P, RB, H, HD // 2], F16, tag="ropep2")
                    nc.vector.tensor_mul(p3[:], x1, tb[2])
                    nc.vector.tensor_mul(p4[:], x2, tb[3])
                    nc.gpsimd.tensor_sub(of[:, :, :, 0:HD // 2], p1[:], p2[:])
                    nc.gpsimd.tensor_add(of[:, :, :, HD // 2:], p3[:], p4[:])
                    # feature-major copy via DMA XBAR transpose
                    for r in range(RB):
                        i = g * RB + r
                        nc.sync.dma_start(qkT[:, :, ts(i, P)], qkR[:, i, :],
                                          transpose=True)

            # ---------------- v (token-major) ------------------------
            vtm = bigp.tile([P, NT, D], F16, tag="big", name="v")
            with nc.named_scope("v"):
                for i in range(NT):
                    for jw in range(NJW):
                        ps = psB.tile([P, JW], F32, tag="psB")
                        for k in range(KT):
                            nc.tensor.matmul(
                                ps[:], latT[:, k, ts(i, P)],
                                wT["v_w"][:, k, ts(jw, JW)],
                                start=(k == 0), stop=(k == KT - 1))
                        nc.vector.tensor_add(vtm[:, i, ts(jw, JW)], ps[:],
                                             bvec[:, BV_V, ts(jw, JW)])

            # ---------------- attention ------------------------------
            # M1: T_h = qk_h^T @ v_h  [HD, HD] per (b, head); head pairs
            # packed into array column halves.  M2: attnT_h = T_h^T @ qkT_h.
            t16s = {}
            with nc.named_scope("attn_m1"):
                for b in range(B_LOC):
                    for hp in range(KT):
                        hA, hB = 2 * hp, 2 * hp + 1
                        pt = psB.tile([P, HD], F32, tag="psB",
                                      name=f"ptm1_{b}_{hp}")
                        for m in range(TPB):
                            mt = b * TPB + m
                            nc.tensor.matmul(
                                pt[0:HD, :],
                                qkR[:, mt, ts(hA, HD)], vtm[:, mt, ts(hA, HD)],
                                start=(m == 0), stop=(m == TPB - 1),
                                tile_position=(0, 0))
                            nc.tensor.matmul(
                                pt[HD:P, :],
                                qkR[:, mt, ts(hB, HD)], vtm[:, mt, ts(hB, HD)],
                                start=(m == 0), stop=(m == TPB - 1),
                                tile_position=(0, HD))
                        t16 = tbp.tile([P, HD], F16, tag="t16",
                                       name=f"t16_{b}_{hp}")
                        nc.scalar.activation(t16[:], pt[:], AF.Copy)
                        t16s[(b, hp)] = t16

            attnT = bigp.tile([P, KT, T], F16, tag="big", name="attnT")
            with nc.named_scope("attn_m2"):
                for b in range(B_LOC):
                    for hp in range(KT):
                        t16 = t16s[(b, hp)]
                        for nw in range(2):
                            col = b * SEQ + nw * TW
                            ps = psA.tile([P, TW], F32, tag="psA")
                            nc.tensor.matmul(
                                ps[0:HD, :], t16[0:HD, :],
                                qkT[0:HD, hp, ds(col, TW)],
                                start=True, stop=True, tile_position=(0, 0))
                            nc.tensor.matmul(
                                ps[HD:P, :], t16[HD:P, :],
                                qkT[HD:P, hp, ds(col, TW)],
                                start=True, stop=True, tile_position=(HD, HD))
                            nc.scalar.activation(attnT[:, hp, ds(col, TW)],
                                                 ps[:], AF.Copy)

            # ------------- gate + output projection + residual -------
            with nc.named_scope("out"):
                for i in range(NT):
                    xr = wk.tile([P, D], F32, tag="xres")
                    nc.sync.dma_start(xr[:], x_flat[ts(i, P), :])
                    for jw in range(NJW):
                        psg = psB.tile([P, JW], F32, tag="psB")
                        for k in range(KT):
                            nc.tensor.matmul(
                                psg[:], xnT[:, k, ts(i, P)],
                                wT["gate_w"][:, k, ts(jw, JW)],
                                start=(k == 0), stop=(k == KT - 1))
                        gt = gwk.tile([P, JW], F16, tag="gt")
                        nc.vector.tensor_add(
                            gt[:], psg[:], bvec[:, BV_GATE, ts(jw, JW)])
                        g16 = gwk.tile([P, JW], F16, tag="g16")
                        nc.scalar.activation(g16[:], gt[:], AF.Sigmoid)

                        ps = psB.tile([P, JW], F32, tag="psB")
                        for k in range(KT):
                            nc.tensor.matmul(
                                ps[:], attnT[:, k, ts(i, P)],
                                wT["out_w"][:, k, ts(jw, JW)],
                                start=(k == 0), stop=(k == KT - 1))
                        ao = gwk.tile([P, JW], F16, tag="ao")
                        nc.vector.tensor_add(
                            ao[:], ps[:], bvec[:, BV_OUT, ts(jw, JW)])
                        nc.vector.tensor_mul(ao[:], ao[:], g16[:])
                        nc.vector.tensor_add(xr[:, ds(jw * JW, JW)], ao[:],
                                             xr[:, ds(jw * JW, JW)])
                    nc.sync.dma_start(out_flat[ts(i, P), :], xr[:])

    nc.finalize()
    return nc


_NC = None


def _get_nc():
    global _NC
    if _NC is None:
        _NC = build_nc()
    return _NC


def make_in_maps(inputs, n_cores=8):
    f32 = np.float32
    x = np.ascontiguousarray(inputs["x"], dtype=f32)
    ln_w = np.asarray(inputs["ln_w"], dtype=f32)
    ln_b = np.asarray(inputs["ln_b"], dtype=f32)

    # per-head output-feature permutation (evens then odds) makes the
    # on-device rope slices contiguous; pure layout prep
    perm = np.concatenate(
        [h * HD + np.concatenate([np.arange(0, HD, 2), np.arange(1, HD, 2)])
         for h in range(H)])

    shared = {}
    # weights: fold LN affine into enc/gate, transpose, cast fp16,
    # flatten to [p, k*D] so each partition line is one contiguous burst
    wmat = {nm: np.asarray(inputs[nm], dtype=f32) for nm in W_NAMES}
    wmat["enc_w"] = wmat["enc_w"] * ln_w[None, :]
    wmat["gate_w"] = wmat["gate_w"] * ln_w[None, :]
    wmat["qk_w"] = wmat["qk_w"][perm]
    for nm in W_NAMES:
        wt = wmat[nm].T.astype(np.float16)            # [d, j]
        wt = wt.reshape(KT, P, D).transpose(1, 0, 2)  # [p, k, j]
        shared[nm] = np.ascontiguousarray(wt.reshape(P, KT * D))

    enc_w = np.asarray(inputs["enc_w"], dtype=f32)
    gate_w = np.asarray(inputs["gate_w"], dtype=f32)
    encb = np.asarray(inputs["enc_b"], dtype=f32) + enc_w @ ln_b
    shared["encb"] = np.ascontiguousarray(encb.reshape(KT, P).T)
    gate_b = np.asarray(inputs["gate_b"], dtype=f32) + gate_w @ ln_b

    bvecs = np.stack([
        np.asarray(inputs["qk_b"], dtype=f32)[perm],
        np.asarray(inputs["v_b"], dtype=f32),
        np.asarray(inputs["out_b"], dtype=f32),
        gate_b,
    ]).astype(np.float16)
    shared["bvecs"] = np.ascontiguousarray(
        np.broadcast_to(bvecs[None], (P, 4, D)))

    # rope tables from rope_emb: host trig, fp16, evens/odds split,
    # pre-scaled so the qk.qk^T product carries 1/sqrt(HD)
    ang = np.asarray(inputs["rope_emb"], dtype=np.float64)[:, :HD]
    cos, sin = np.cos(ang) * QK_SCALE, np.sin(ang) * QK_SCALE
    tabs = np.stack([cos[:, 0::2], sin[:, 0::2], sin[:, 1::2], cos[:, 1::2]],
                    axis=1)                          # [N, 4, 32]
    tabs = tabs.reshape(TPB, P, 4, HD // 2).transpose(1, 0, 2, 3)
    shared["rope_tabs"] = np.ascontiguousarray(tabs.astype(np.float16))

    in_maps = []
    for c in range(n_cores):
        m = dict(shared)
        m["x"] = np.ascontiguousarray(x[c * B_LOC:(c + 1) * B_LOC])
        in_maps.append(m)
    return in_maps


def kernel(**inputs):
    nc = _get_nc()
    n_cores = 8
    in_maps = make_in_maps(inputs, n_cores)
    res = bass_utils.run_bass_kernel_spmd(
        nc, in_maps, core_ids=list(range(n_cores)))
    return np.concatenate([r["out"] for r in res.results], axis=0)
